# revision 52
# baseline (speedup 1.0000x reference)
"""Equivariant MLP (9 -> 49 -> 49 -> 9, tied weights) on 8 trn2 NeuronCores.

Data parallel over the batch (1048576 rows -> 131072/core).  Tied-weight
patterns are expanded to dense matrices on the host.  Samples are processed
in PAIRS: x^T arrives as [19, 65536] bf16 (rows 0-17 = two samples' features
stacked, row 18 = ones) and every layer's bias is folded into the matmul via
the ones row, which each weight matrix propagates (extra unit column) so no
engine ever adds a bias.

Per 1024-pair iteration:
  L1  PE   [19,99]w  x [19,1024]   -> psum1 [99,1024]   (bias via ones row)
  h1  ACT  relu(psum1) -> sbuf bf16 [99,1024]           (ones row survives)
  L2  PE   [99,99]w x h1           -> psum2 [99,1024]
  h2  DVE  max(psum2,0) -> sbuf bf16 [99,1024]
  L3  PE   FLIPPED: stationary = h2 128-col chunk, moving = [99,18] weights
           -> psum3 [128 pairs, 18] per chunk; 8 chunks = [128,144].
           Ldweights is free, so L3 costs 144 PE columns instead of 1024.
  y   ACT  copy psum3 -> sbuf f32, DMA out every 8 iterations.

Emission is software-pipelined (stages: L1 at i, h1/L2/h2 at i-1, L3 at
i-4, y at i-5) so no in-order engine queue waits behind a dependent op.
PSUM (8 banks): p1/p2 share a 3-slot rotation (6 banks); p3 has its own
2-slot tag (1 bank each) holding y for [3,3,2]-iteration groups so ACT
does only 3 y-copies per 8 iterations.  All x chunks are preloaded into
dedicated SBUF buffers; junk matmuls during the initial DMA wait pre-ramp
the PE out of its low p-state.

Startup: the weight DMA is row-split — rows 0:19 (w1 plus the ENTIRE first
round's 1024 x columns) land first and alone gate L1(0); x chunk 0's bulk
and the w2/w3 rows follow (DEFAULT_OPT wp_x1024).  Tail: round 63's h2
runs on DVE, its L3 writes a private [128,144] p3 tile (the Tile
framework's dependency tracking is TILE-granular, so sharing the [2]-group
tile would stall L3(63) behind the q=6 y-copy), and the final y piece is
copied by the otherwise-idle DVE; the last DMA is a quarter-group.

Steady state (TimelineSim 89421ns/core): DVE is the pacer — ACT and DVE
are the only engines that can touch PSUM (GPSIMD is verifier-rejected,
DMA can't address PSUM, 16-bit PSUM matmul out is TRN3-only), so the
2048+144 psum columns per round are split h1->ACT (1038ns) / h2->DVE
(1192ns) with y copies riding ACT's 154ns/round slack.

Multi-wait splitting policy (keep_wait, worth ~1.2us): walrus only allows
one sync-wait per instruction, so extra waits are parked on
sequencer-blocking NoOps.  WHICH wait stays on the instruction matters:
the kept wait is checked in the engine wait queue (pre-staging continues)
while parked waits stall the whole sequencer.  Empirically DVE wants its
FIRST-listed wait kept (its h2's late wait is the second L2 matmul,
listed first), PE and SP do best with a sem-count-prefix "smart" ranking,
ACT with last.  Steady y-groups phased (2,3,3) beat (3,3,2) slightly.

Startup hoist (hoist_dma/hoist_pre_rm, worth ~820ns): the two
startup-critical SP DMAs carry no waits, so a post-pass moves them from
the main block to the very top of SP's preamble stream — ahead of the
RegisterMoves and the ~700ns EventSemaphore range-clear.  Their
completion semaphores fire ~1us after the clear finishes, so the clear
cannot erase them, and their access patterns are static (no register
references), so running before the RegisterMoves is safe.  With the
earlier start, round-0's h1 split across ACT||DVE in separate tiles
(h1ab) also pays off; hoisting a third DMA does not (the extra HWDGE
hold delays the critical first one).  Swapping evac engines on
copy-burdened rounds (to land bursts on the slack-edged tensor) breaks
the psum rotation pairing and loses 10.8us — the h1->ACT/h2->DVE
assignment is load-bearing for the slot cycle.  The exit block carries
TWO per-engine Drain+EventSemaphore barrier rounds; the second is
redundant and stripped (-261ns), but the FIRST is load-bearing — with
both removed the outputs corrupt (rel err 0.95), so the DMA-guard NoOps
alone do not order output landing.

Measured dead ends: moving any y copy to DVE (+212ns each), splitting any
evac in half (+185/125 fixed per extra instruction AND tile-granular deps
make the halves wait on both matmuls anyway), explicit psum slot pinning
(+2.4us vs the shared-tag rotation), emitting L2 one round later (l2lag —
provably zero effect: h1's completion time is physical, not an emission
artifact), per-3-round y groups with per-group DMAs, engine-swapping
h1<->h2, junk-matmul p-state bridges, and issuing DMAs from other queues.
"""

import os
import sys

sys.path.insert(0, "/opt/trn_rl_repo")

import numpy as np
import ml_dtypes

import concourse.bass as bass
import concourse.mybir as mybir
import concourse.tile as tile
from concourse.bass_utils import run_bass_kernel_spmd

f32 = mybir.dt.float32
bf16 = mybir.dt.bfloat16

N_CORES = 8
BATCH = 1048576
BS = BATCH // N_CORES          # 131072 samples per core
NPAIR = BS // 2                # 65536 pair columns per core
C = 1024                       # pair columns per iteration
NITER = NPAIR // C             # 64
XCH = 8                        # iterations per x DMA chunk
YCH = 8                        # iterations per y DMA chunk
MM = 512                       # matmul moving width (one PSUM bank)

last_exec_ns = None


def _split_multi_waits(nc, keep="last"):
    """Walrus in this container rejects instructions carrying more than one
    sync wait ("Too many sync wait commands").  Re-park all but one wait of
    every instruction on same-engine NoOps inserted just before it.

    The kept wait stays on the instruction (checked in the engine WAIT_QUEUE,
    overlapping with SEQ decode of later instructions); parked waits block
    the sequencer itself.  Which wait fires last therefore matters."""
    n = 0
    for fn in nc.m.functions:
        for bb in fn.blocks:
            out = []
            counts = {}

            def slack(w):
                # How long ago (in emitted sem updates) this wait's target
                # was reached: big = fires early, negative = the signalling
                # instruction hasn't even been emitted yet (fires latest).
                have = counts.get((str(w.sync_type), w.id), 0)
                need = w.wait_value if w.wait_value is not None else 0
                return have - need

            for inst in bb.instructions:
                si = inst.sync_info
                waits = list(si.on_wait) if (si and si.on_wait) else []
                if len(waits) > 1:
                    kv = keep
                    if isinstance(keep, dict):
                        okey = f"{inst.engine}:{type(inst).__name__}"
                        fn = getattr(inst, "func", None)
                        fkey = f"{inst.engine}:{fn}" if fn is not None else None
                        kv = keep.get(fkey or "",
                                      keep.get(okey,
                                               keep.get(str(inst.engine),
                                                        "last")))
                    if kv == "first":
                        keep_w, park = waits[:1], waits[1:]
                    elif kv == "smart":
                        # keep the latest-firing wait ON the instruction
                        # (checked in the engine wait queue, no SEQ stall);
                        # park early-firing waits on NoOps, earliest first,
                        # so the sequencer sails through them.
                        order = sorted(waits, key=slack)
                        keep_w = [order[0]]
                        park = sorted(order[1:], key=slack, reverse=True)
                    else:
                        keep_w, park = waits[-1:], waits[:-1]
                    si.on_wait = keep_w
                    for w in park:
                        nop = mybir.InstNoOp(name=f"WSPLIT-{n}", ins=[], outs=[])
                        n += 1
                        nop.engine = inst.engine
                        nop.sync_info = mybir.SyncInfo(on_update=[], on_wait=[w])
                        out.append(nop)
                out.append(inst)
                if si and si.on_update:
                    for u in si.on_update:
                        key = (str(u.sync_type), u.id)
                        mode = str(u.update_mode)
                        val = u.update_value if u.update_value is not None else 1
                        if "wr" in mode:
                            counts[key] = val
                        else:
                            counts[key] = counts.get(key, 0) + val
            bb.instructions = out
    return nc


DEFAULT_OPT = {
    "wp_x1024": True,   # first round's x rides the (row-split) weight DMA
    "tail_p3": True,    # final round gets a private p3 tile (no WAR stall)
    "tail_dve": True,   # final h2 on DVE
    "q7_dve": True,     # final y piece copied by DVE
    # which of a multi-wait instruction's waits stays ON the instruction
    # (the others get parked on sequencer-blocking NoOps): keeping DVE's
    # FIRST-listed wait unblocks the DVE sequencer's pre-staging and is
    # worth ~1us.
    "keep_wait": {"EngineType.DVE": "first", "EngineType.PE": "smart",
                  "EngineType.SP": "smart"},
    "ygroups": (2, 3, 3),  # steady-frame y grouping (tail frame stays 3,3,2)
    "hoist_dma": 2,     # start the first two SP DMAs before SP's preamble
                        # EventSemaphore clear (~700ns earlier pipeline start)
    "hoist_pre_rm": True,  # ...and ahead of SP's RegisterMoves (-275ns more)
    "h1ab": True,       # round-0 h1 split across ACT||DVE in separate tiles
    "strip_exit": 13,   # keep the first exit barrier round, drop the second
                        # (dropping both corrupts outputs: rel err 0.95)
}


def _build_nc(**opt):
    opt = {**DEFAULT_OPT, **opt}
    WX = 1024 if opt.get("wp_x1024", False) else MM
    nc = bass.Bass()
    xt = nc.dram_tensor("xt", [19, NPAIR], bf16, kind="ExternalInput")
    # wp cols 0:99 = w2e, 99:117 = w3e, 117:216 = w1e (rows 0:19),
    # 216:216+WX = first WX x columns (rows 0:19) — one startup DMA
    wp = nc.dram_tensor("wp", [99, 216 + WX], bf16, kind="ExternalInput")
    yt = nc.dram_tensor("yt", [NITER // YCH, 128, YCH * 144], f32,
                        kind="ExternalOutput")

    relu = mybir.ActivationFunctionType.Relu
    amax = mybir.AluOpType.max
    XW = XCH * C                   # pair columns per x chunk

    with tile.TileContext(nc) as tc:
        with (
            tc.tile_pool(name="consts", bufs=1) as cp,
            tc.tile_pool(name="xp", bufs=2) as xp,
            tc.tile_pool(name="hid", bufs=2) as hp,
            tc.tile_pool(name="yp", bufs=2) as yp,
            tc.tile_pool(name="ps", bufs=3, space=bass.MemorySpace.PSUM) as pp,
        ):
            wpt = cp.tile([99, 216 + WX], bf16)
            w2t = wpt[:, 0:99]
            w3t = wpt[:, 99:117]
            w1t = wpt[0:19, 117:216]

            xts = {}

            XCHo = opt.get("xch", XCH)
            XWo = XCHo * C

            def xdma(g):
                xts[g] = xp.tile([19, XWo], bf16, tag="xts",
                                 bufs=NITER // XCHo, name=f"xts{g}")
                if g == 0 and opt.get("wp_x1024", False):
                    # the full first round's x rides in the wp DMA; only the
                    # bulk remains
                    nc.sync.dma_start(xts[g][:, C:XWo], xt[:, C:XWo])
                elif g == 0:
                    # first 512 x cols ride in the wp DMA; a small DMA brings
                    # iteration 0's second half early, then the bulk
                    nc.sync.dma_start(xts[g][:, MM:C], xt[:, MM:C])
                    nc.sync.dma_start(xts[g][:, C:XWo], xt[:, C:XWo])
                else:
                    nc.sync.dma_start(xts[g][:], xt[:, g * XWo:(g + 1) * XWo])

            if opt.get("wp_x1024", False):
                # rows 0:19 (w1 + the whole first round's x) go first — they
                # alone gate L1(0); then x chunk 0's bulk; then the w2/w3
                # rows; then the remaining x chunks
                nc.sync.dma_start(wpt[0:19, 117:216 + WX],
                                  wp[0:19, 117:216 + WX])
                xdma(0)
                nc.sync.dma_start(wpt[:, 0:117], wp[:, 0:117])
                for g in range(1, NITER // XCHo):
                    xdma(g)
            elif opt.get("wp_split", False):
                # rows 0:19 (w1 + first x cols) first — it alone gates L1(0);
                # the w2/w3 rows follow in a second, cheap DMA
                nc.sync.dma_start(wpt[0:19, 117:216 + MM],
                                  wp[0:19, 117:216 + MM])
                nc.sync.dma_start(wpt[:, 0:117], wp[:, 0:117])
            elif opt.get("wp_act", False):
                # ACT's sequencer finishes the TileContext preamble ~1us
                # before SP's, so launching the weight DMA there starts the
                # whole pipeline earlier
                nc.scalar.dma_start(wpt[:], wp[:])
            else:
                nc.sync.dma_start(wpt[:], wp[:])
            if not opt.get("wp_x1024", False):
                for g in range(NITER // XCHo):
                    xdma(g)

            # Warm the PE p-state during the initial DMA wait: junk matmuls
            # on a never-written private tile (uninitialized reads are fine —
            # the psum result is discarded and the real L3 resets the slot
            # with start=True).  No memset, no WAR: the p-state clock starts
            # at the PE prologue (~0.55us) so the first real matmul at ~3.6us
            # is already past the 3us full-ramp threshold.
            jw = min(opt.get("junk_w", 8), 432)
            junk = cp.tile([1, max(jw, 8)], bf16)
            nc.vector.memset(junk[:], 0.0)
            jp = pp.tile([128, 432], f32, tag="p3", bufs=2, name="jp")
            for _ in range(opt.get("junk", 4)):
                nc.tensor.matmul(jp[0:1, 0:jw], junk[0:1, 0:1],
                                 junk[0:1, 0:jw], start=True, stop=True)

            p1s, p2s, p3s, h1s, h2s = {}, {}, {}, {}, {}
            ycur = [None]
            ylast = [None]

            def stage_L1(i):
                if not (i < NITER):
                    return
                p1 = pp.tile([99, C], f32, tag="ps", bufs=3,
                             name=f"p1_{i}")
                if i == 0:
                    nc.tensor.matmul(p1[:, 0:MM], w1t[:],
                                     wpt[0:19, 216:216 + MM],
                                     start=True, stop=True)
                    if opt.get("wp_x1024", False):
                        nc.tensor.matmul(p1[:, MM:C], w1t[:],
                                         wpt[0:19, 216 + MM:216 + C],
                                         start=True, stop=True)
                    else:
                        nc.tensor.matmul(p1[:, MM:C], w1t[:],
                                         xts[0][:, MM:C],
                                         start=True, stop=True)
                else:
                    src = xts[i // XCHo]
                    off = (i % XCHo) * C
                    for m in range(0, C, MM):
                        nc.tensor.matmul(
                            p1[:, m:m + MM], w1t[:],
                            src[:, off + m:off + m + MM],
                            start=True, stop=True)
                p1s[i] = p1

            # y iterations grouped per 8-iter chunk: a PSUM bank holds at
            # most 3 iterations' worth ([128, 432]), so 3 copies/chunk.
            # Steady frames may use a different 8=a+b+c split; the tail
            # frame always uses [3,3,2] (its flush logic assumes it).
            def ygrp(m):
                if m == NITER // YCH - 1:
                    sizes = (3, 3, 2)
                else:
                    sizes = opt.get("ygroups", (3, 3, 2))
                starts, s = [], 0
                for sz in sizes:
                    starts.append(s)
                    s += sz
                gstart = {st: g for g, st in enumerate(starts)}
                glast = {st + sz - 1: g
                         for g, (st, sz) in enumerate(zip(starts, sizes))}
                gw = tuple(sz * 144 for sz in sizes)
                c0s = tuple(st * 144 for st in starts)
                return gstart, glast, gw, c0s

            GSTART = {0: 0, 3: 1, 6: 2}          # tail-frame view (3,3,2)
            GLAST = {2: 0, 5: 1, 7: 2}
            GW = (432, 432, 288)

            def stage_y(i):
                # y-copy for the group that finished at round i-5.  Its deps
                # are rounds old, so ACT runs it while it would otherwise
                # idle waiting for this round's psum.
                yoff = opt.get("yoff", 5)
                if not (yoff <= i <= NITER + yoff - 1):
                    return
                k = i - yoff
                m = k // YCH
                if m == NITER // YCH - 1:
                    return                       # tail handled in stage_L3
                gstart, glast, gw, c0s = ygrp(m)
                if k % YCH not in glast:
                    return
                go = glast[k % YCH]
                gid = m * 3 + go
                w = gw[go]
                if go == 0:
                    ycur[0] = yp.tile([128, YCH * 144], f32,
                                      tag="yt", bufs=NITER // YCH,
                                      name=f"y_{m}")
                c0 = c0s[go]
                dst = ycur[0][:, c0:c0 + w]
                if go == 2 and m in opt.get("g2_dve_frames", ()):
                    nc.vector.tensor_copy(dst, p3s.pop(gid)[:, 0:w])
                else:
                    nc.scalar.copy(dst, p3s.pop(gid)[:, 0:w])
                if go == 2:
                    nc.sync.dma_start(yt[m], ycur[0][:])

            def stage_h1(i):
                if not (1 <= i <= NITER):
                    return
                k = i - 1
                if k == 0 and opt.get("h1ab", False):
                    # startup: round 0's h1 in two separate tiles on ACT
                    # and the (still idle) DVE concurrently; separate tiles
                    # because dependency tracking is tile-granular, and
                    # L2(0)'s two matmuls then each wait only their half
                    h1a = hp.tile([99, MM], bf16, tag="h1a", bufs=1,
                                  name="h1a0")
                    h1b = hp.tile([99, MM], bf16, tag="h1b", bufs=1,
                                  name="h1b0")
                    nc.scalar.activation(h1a[:], p1s[k][:, 0:MM], relu)
                    nc.vector.tensor_scalar(h1b[:], p1s.pop(k)[:, MM:C],
                                            0.0, None, amax)
                    h1s[k] = ("ab", h1a, h1b)
                    return
                h1 = hp.tile([99, C], bf16, tag="h1",
                             bufs=opt.get("h1bufs", 2), name=f"h1_{k}")
                if k in swapR or opt.get("swap_engines", False):
                    nc.vector.tensor_scalar(h1[:], p1s.pop(k)[:], 0.0,
                                            None, amax)
                else:
                    nc.scalar.activation(h1[:], p1s.pop(k)[:], relu)
                h1s[k] = h1

            def stage_L2(i, lag):
                k = i - lag
                if not (0 <= k < NITER):
                    return
                p2 = pp.tile([99, C], f32, tag="ps", bufs=3,
                             name=f"p2_{k}")
                h1 = h1s.pop(k)
                if isinstance(h1, tuple):
                    for m, part in ((0, h1[1]), (MM, h1[2])):
                        nc.tensor.matmul(
                            p2[:, m:m + MM], w2t[:], part[:],
                            start=True, stop=True)
                else:
                    for m in range(0, C, MM):
                        nc.tensor.matmul(
                            p2[:, m:m + MM], w2t[:], h1[:, m:m + MM],
                            start=True, stop=True)
                p2s[k] = p2

            def stage_h2(i, lag):
                k = i - lag
                if not (0 <= k < NITER):
                    return
                h2 = hp.tile([99, C], bf16, tag="h2",
                             bufs=opt.get("h2bufs", 4), name=f"h2_{k}")
                p2 = p2s.pop(k)
                if k == NITER - 1 and not opt.get("tail_dve", False):
                    # tail: ACT is idle after its last h1 and is faster
                    nc.scalar.activation(h2[:], p2[:], relu)
                elif (k in swapR and k != NITER - 1) or opt.get("swap_engines", False):
                    nc.scalar.activation(h2[:], p2[:], relu)
                else:
                    nc.vector.tensor_scalar(h2[:], p2[:], 0.0, None, amax)
                h2s[k] = h2

            def stage_L3(i):
                if not (4 <= i <= NITER + 3):
                    return
                k = i - 4
                q = k % YCH
                tailf = (k // YCH == NITER // YCH - 1
                         and opt.get("tail_p3", False))
                gstart, glast, gw, c0s = ygrp(k // YCH)
                if q in gstart:
                    gid = (k // YCH) * 3 + gstart[q]
                    p3s[gid] = pp.tile([128, 432], f32, tag="p3",
                                       bufs=2, name=f"p3_{gid}")
                gid = (k // YCH) * 3 + [g for r, g in gstart.items()
                                        if r <= q][-1]
                h2 = h2s.pop(k)
                if tailf and q == 7:
                    # final round gets its own p3 tile so this L3 does not
                    # inherit a tile-granular WAR on the q=6 y-copy
                    p3s["t63"] = pp.tile([128, 144], f32, tag="p3",
                                         bufs=2, name="p3_t63")
                    p3 = p3s["t63"]
                    base = 0
                else:
                    p3 = p3s[gid]
                    base = (q - [r for r in gstart if r <= q][-1]) * 144
                for c in range(8):
                    nc.tensor.matmul(
                        p3[:, base + c * 18:base + (c + 1) * 18],
                        h2[:, c * 128:(c + 1) * 128], w3t[:],
                        start=True, stop=True)
                if k // YCH == NITER // YCH - 1 and q in (2, 5, 6, 7):
                    # tail: copy + DMA per completed piece; the final group
                    # flushes per-iteration so the last DMA is a quarter
                    if ylast[0] is None:
                        ylast[0] = yp.tile([128, YCH * 144], f32,
                                           tag="yt", bufs=NITER // YCH,
                                           name="ylast")
                    if q in (2, 5):
                        go = GLAST[q]
                        c0, w = (0, 432, 864)[go], 432
                        src = p3s.pop(gid)[:, 0:w]
                    elif q == 6:
                        c0, w = 864, 144
                        src = p3s[gid][:, 0:144]
                    elif tailf:
                        c0, w = 1008, 144
                        p3s.pop(gid)
                        src = p3s.pop("t63")[:, 0:144]
                    else:
                        c0, w = 1008, 144
                        src = p3s.pop(gid)[:, 144:288]
                    dst = ylast[0][:, c0:c0 + w]
                    if q == 7 and opt.get("q7_dve", False):
                        nc.vector.tensor_copy(dst, src)
                    else:
                        nc.scalar.copy(dst, src)
                    nc.sync.dma_start(yt[k // YCH, :, c0:c0 + w], dst)

            # rounds whose ACT y-copy burst would otherwise delay that
            # round's h1: swap the evac engines there so the burst lands on
            # h2 (whose consumers have rounds of slack) instead
            swapR = set()
            if opt.get("swap_rounds", False):
                sizes = opt.get("ygroups", (3, 3, 2))
                for m in range(NITER // YCH - 1):
                    s = 0
                    for sz in sizes:
                        ge = m * YCH + s + sz - 1
                        if ge + 4 < NITER - 1:
                            swapR.add(ge + 4)
                        s += sz

            lag = opt.get("l2lag", 1)
            for i in range(NITER + 6):
                if lag >= 2:
                    # L2/h2 consume a round-old h1: ACT's y-copy bursts get
                    # a full round of decoupling before they can reach the
                    # DVE pacer.  L2 is emitted BEFORE L1 so p2(k) reuses
                    # p1(k)'s psum slot (whose WAR wait is data-redundant).
                    stage_L2(i, lag)
                    stage_h2(i, lag)
                    stage_L1(i)
                    stage_y(i)
                    stage_h1(i)
                elif i in opt.get("early_l2", ()):
                    # startup: let L2(i-1) ahead of L1(i) in PE's in-order
                    # queue so h2(i-1) isn't delayed behind a prefetch L1
                    stage_y(i)
                    stage_h1(i)
                    stage_L2(i, 1)
                    stage_h2(i, 1)
                    stage_L1(i)
                else:
                    stage_L1(i)
                    stage_y(i)
                    stage_h1(i)
                    stage_L2(i, 1)
                    stage_h2(i, 1)
                stage_L3(i)

    if opt.get("hoist_dma", 0):
        # Hoist the first startup-critical SP DMAs (they carry no waits)
        # from the main block to before SP's ~700ns preamble EventSemaphore
        # range-clear: their completion sems fire only ~1us AFTER the clear
        # finishes, so the clear cannot erase them, and the whole pipeline
        # starts one clear earlier.
        blocks = nc.m.functions[0].blocks
        b0, b1 = blocks[0], blocks[1]
        if opt.get("hoist_pre_rm", False):
            # even earlier: ahead of SP's RegisterMoves (the DMA APs are
            # static, no register references)
            spev = next(i for i, inst in enumerate(b0.instructions)
                        if str(inst.engine) == "EngineType.SP")
        else:
            spev = next(i for i, inst in enumerate(b0.instructions)
                        if type(inst).__name__ == "InstEventSemaphore"
                        and str(inst.engine) == "EngineType.SP")
        moved, kept = [], []
        for inst in b1.instructions:
            if (len(moved) < opt["hoist_dma"]
                    and type(inst).__name__ == "InstDMACopy"
                    and str(inst.engine) == "EngineType.SP"
                    and not (inst.sync_info and inst.sync_info.on_wait)):
                moved.append(inst)
            else:
                kept.append(inst)
        b1.instructions = kept
        b0.instructions = (b0.instructions[:spev] + moved
                           + b0.instructions[spev:])
    if opt.get("strip_exit", 0):
        # The exit block carries TWO full per-engine Drain+EventSemaphore
        # barrier rounds; the second is redundant for completion (the DMA
        # guard NoOps + first barrier already order everything).
        bb = nc.m.functions[0].blocks[-1]
        drop = opt["strip_exit"]
        keepn = []
        seen = 0
        for inst in bb.instructions:
            tn = type(inst).__name__
            if tn in ("InstDrain", "InstEventSemaphore", "InstISA"):
                seen += 1
                if seen > drop:
                    continue
            keepn.append(inst)
        bb.instructions = keepn
    return _split_multi_waits(nc, keep=opt.get("keep_wait", "last"))


_nc_cache = {}


def _get_nc(*_ignored):
    if "nc" not in _nc_cache:
        _nc_cache["nc"] = _build_nc()
    return _nc_cache["nc"]


def _expand(pattern, params):
    pattern = np.asarray(pattern)
    params = np.asarray(params, np.float32)
    return np.where(pattern > 0, params[np.maximum(pattern - 1, 0)], 0.0).astype(
        np.float32
    )


def _blockdiag(a):
    o = np.zeros((2 * a.shape[0], 2 * a.shape[1]), np.float32)
    o[:a.shape[0], :a.shape[1]] = a
    o[a.shape[0]:, a.shape[1]:] = a
    return o


def kernel(**inputs):
    global last_exec_ns
    x = np.ascontiguousarray(np.asarray(inputs["x"], np.float32))
    W1 = _expand(inputs["wp1"], inputs["w1"])  # [9, 49]
    W2 = _expand(inputs["wp2"], inputs["w2"])  # [49, 49]
    W3 = _expand(inputs["wp3"], inputs["w3"])  # [49, 9]
    B1 = _expand(inputs["bp1"], inputs["b1"])  # [49]
    B2 = _expand(inputs["bp2"], inputs["b2"])  # [49]
    B3 = _expand(inputs["bp3"], inputs["b3"])  # [9]

    w1e = np.zeros((19, 99), np.float32)
    w1e[0:18, 0:98] = _blockdiag(W1)
    w1e[18, 0:98] = np.concatenate([B1, B1])
    w1e[18, 98] = 1.0
    w2e = np.zeros((99, 99), np.float32)
    w2e[0:98, 0:98] = _blockdiag(W2)
    w2e[98, 0:98] = np.concatenate([B2, B2])
    w2e[98, 98] = 1.0
    w3e = np.zeros((99, 18), np.float32)
    w3e[0:98, :] = _blockdiag(W3)
    w3e[98, :] = np.concatenate([B3, B3])

    WXH = 1024 if DEFAULT_OPT.get("wp_x1024", False) else MM
    wpk = np.zeros((99, 216 + WXH), np.float32)
    wpk[:, 0:99] = w2e
    wpk[:, 99:117] = w3e
    wpk[0:19, 117:216] = w1e

    ones = np.ones((1, NPAIR), np.float32)
    in_maps = []
    for c in range(N_CORES):
        xc = x[c * BS:(c + 1) * BS]                       # [BS, 9]
        xpair = xc.reshape(NPAIR, 18).T                   # [18, NPAIR] view
        xfull = np.concatenate([xpair, ones], axis=0)     # [19, NPAIR]
        wpc = wpk.copy()
        wpc[0:19, 216:216 + WXH] = xfull[:, 0:WXH]
        in_maps.append({
            "xt": np.ascontiguousarray(xfull).astype(ml_dtypes.bfloat16),
            "wp": wpc.astype(ml_dtypes.bfloat16),
        })

    nc = _get_nc()
    trace = os.environ.get("KERNEL_TRACE", "0") == "1"
    # The axon-tunneled NRT intermittently fails with
    # NRT_EXEC_UNIT_UNRECOVERABLE; a plain retry recovers it.
    last_err = None
    for attempt in range(4):
        try:
            res = run_bass_kernel_spmd(
                nc, in_maps, core_ids=list(range(N_CORES)), trace=trace
            )
            break
        except Exception as e:  # noqa: BLE001
            last_err = e
            import time as _time

            _time.sleep(2.0 * (attempt + 1))
    else:
        raise last_err
    if trace:
        last_exec_ns = res.exec_time_ns

    y = np.empty((BATCH, 9), np.float32)
    for c in range(N_CORES):
        ytc = res.results[c]["yt"]  # [8, 128, 1152]
        # ytc[g, n, q*144 + cc*18 + h*9 + f] -> sample 2*(((g*8+q)*8+cc)*128+n)+h
        arr = ytc.reshape(8, 128, YCH, 8, 2, 9).transpose(0, 2, 3, 1, 4, 5)
        y[c * BS:(c + 1) * BS] = arr.reshape(BS, 9)
    return y
